# revision 4
# baseline (speedup 1.0000x reference)
"""Trainium2 Bass kernel for ExcitationEmbedding + Ion RoPE.

Computes, for inputs
  excitations [256, 512, 2] int64 (pairs (a, b) with a, b in [0, 6)),
  n_electrons [256] f32, n_protons [256] f32,
  emb_weight  [26, 256] f32, lookup_table [6, 6] int64:

  idx   = lookup_table[a, b]                       # [B, N]
  emb   = emb_weight[idx]                          # [B, N, D]
  out   = per-batch block-diagonal rotation of emb (theta from n_electrons,
          phi from n_protons, 4-wide blocks: dims (0,1) by theta, (2,3) by phi)

Strategy (v2; pure data parallel over 8 cores, 32 batches each):
  - Host sends flat codes f = 6*a + b as int8; the device builds one-hot
    rows against an iota via is_equal (the 26-row table and 6x6 lut are
    consumed on-device through a select-matmul that produces emb36).
  - Per batch, the rotated 36-row table rot[j, d] is built on-chip from
    per-batch cos/sin scalars (partition-broadcast via a tiny DRAM bounce)
    and split into an fp8-e4m3 hi + residual lo pair (error feedback).
  - The gather runs transposed on the PE: out_T[d_half, tok] =
    lhsT[hi;lo planes].T @ onehot with fp8 DoubleRow (0.5 cycles/row),
    streaming all 512 tokens per matmul; 2 matmuls + 1 weight tile/batch.
  - Output is written as fp16 in a [128, BL, 2, 512] d-major DRAM layout
    (fully linear 8 KB per-partition packets); the host transposes back
    and converts to f32.
"""

import functools

import numpy as np

import concourse.bass as bass
import concourse.bacc as bacc
import concourse.mybir as mybir
from concourse import tile
from concourse.bass_utils import run_bass_kernel_spmd

B, N, D = 256, 512, 256
N_CORES = 8
BL = B // N_CORES   # 32 batches per core
G = 4               # batches per output DMA group
ANGLE_SCALE = 0.05
HALF_PI = float(np.pi / 2)

F32 = mybir.dt.float32
F16 = mybir.dt.float16
I8 = mybir.dt.int8
FP8 = mybir.dt.float8e4
AF = mybir.ActivationFunctionType
ALU = mybir.AluOpType
DR = mybir.MatmulPerfMode.DoubleRow


def build_bass() -> bass.Bass:
    nc = bacc.Bacc(
        "TRN2", target_bir_lowering=False, debug=False, num_devices=N_CORES
    )

    flat_in = nc.dram_tensor("flat", [1, BL * N], I8, kind="ExternalInput")
    ne = nc.dram_tensor("ne", [BL, 1], F32, kind="ExternalInput")
    npr = nc.dram_tensor("npr", [BL, 1], F32, kind="ExternalInput")
    emb = nc.dram_tensor("emb", [26, D], F32, kind="ExternalInput")
    lut = nc.dram_tensor("lut", [1, 36], F32, kind="ExternalInput")
    # out[p, b, h, n] = result[b, n, h*128 + p]
    out = nc.dram_tensor("out", [128, BL * 2 * N], F16, kind="ExternalOutput")

    iota_f32 = nc.inline_tensor(
        np.arange(36, dtype=np.float32).reshape(36, 1), "iota_f32")

    with tile.TileContext(nc) as tc:
        with (
            tc.tile_pool(name="const", bufs=1) as const,
            tc.tile_pool(name="bpool", bufs=3) as bpool,
            tc.tile_pool(name="opool", bufs=2) as opool,
            tc.tile_pool(name="dram", bufs=1, space="DRAM") as dram,
            tc.tile_pool(name="psum_s", bufs=1, space="PSUM") as psum_s,
            tc.tile_pool(name="psum", bufs=6, space="PSUM") as psum,
        ):
            # ---- loads (scalar HWDGE queue; sync carries output writes) ----
            ne_s = const.tile([BL, 1], F32)
            nc.scalar.dma_start(out=ne_s[:], in_=ne[:])
            npr_s = const.tile([BL, 1], F32)
            nc.scalar.dma_start(out=npr_s[:], in_=npr[:])
            emb_f = const.tile([26, D], F32)
            nc.scalar.dma_start(out=emb_f[:], in_=emb[:])
            lut_bc = const.tile([26, 36], F32)
            nc.scalar.dma_start(out=lut_bc[:],
                                in_=lut[0:1, :].to_broadcast((26, 36)))
            flat_bc = const.tile([36, BL * N], I8)
            nc.scalar.dma_start(out=flat_bc[:],
                                in_=flat_in[0:1, :].to_broadcast((36, BL * N)))
            iota_s = const.tile([36, 1], F32)
            nc.scalar.dma_start(out=iota_s[:], in_=iota_f32[:])

            # ---- per-batch angle scalars: [ct ct cp cp st -st sp -sp] ----
            hp = const.tile([BL, 1], F32)
            nc.vector.memset(hp[:], HALF_PI)
            scal8 = const.tile([BL, 8], F32)
            # cos(t) = sin(pi/2 - t) keeps the LUT argument within [-pi, pi]
            specs = [
                (ne_s, True, -ANGLE_SCALE), (ne_s, True, -ANGLE_SCALE),
                (npr_s, True, -ANGLE_SCALE), (npr_s, True, -ANGLE_SCALE),
                (ne_s, False, ANGLE_SCALE), (ne_s, False, -ANGLE_SCALE),
                (npr_s, False, ANGLE_SCALE), (npr_s, False, -ANGLE_SCALE),
            ]
            for i, (src, use_hp, scale) in enumerate(specs):
                nc.scalar.activation(scal8[:, i:i + 1], src[:], AF.Sin,
                                     bias=hp[:] if use_hp else 0.0, scale=scale)
            scal_d = dram.tile([BL, 8], F32)
            nc.sync.dma_start(out=scal_d[:], in_=scal8[:])
            scal_bc = const.tile([36, BL * 8], F32)
            nc.sync.dma_start(
                out=scal_bc[:],
                in_=scal_d[:].rearrange("q c -> (q c)").unsqueeze(0)
                .to_broadcast((36, BL * 8)))

            # ---- emb36 via select-matmul; pair-swapped copy ----
            emb16 = const.tile([26, D], F16)
            nc.vector.tensor_copy(emb16[:], emb_f[:])
            selT = const.tile([26, 36], F16)
            nc.vector.tensor_scalar(out=selT[:], in0=lut_bc[:],
                                    scalar1=iota_s[0:26, :], scalar2=None,
                                    op0=ALU.is_equal)
            eph_ps = psum_s.tile([36, D], F32)
            nc.tensor.matmul(eph_ps[:], selT[:], emb16[:], start=True, stop=True)
            e16 = const.tile([36, D], F16)
            nc.scalar.activation(e16[:], eph_ps[:], AF.Copy)
            esw = const.tile([36, D], F16)
            e2 = e16[:].rearrange("j (k i) -> j k i", i=2)
            s2 = esw[:].rearrange("j (k i) -> j k i", i=2)
            nc.vector.tensor_copy(s2[:, :, 0], e2[:, :, 1])
            nc.vector.tensor_copy(s2[:, :, 1], e2[:, :, 0])
            ones64 = const.tile([36, 64], F16)
            nc.vector.memset(ones64[:], 1.0)

            obuf = None
            for b in range(BL):
                # ---- rotation patterns from scalars (DVE + gpsimd) ----
                cpat = bpool.tile([36, D], F16, tag="cpat", bufs=3)
                spat = bpool.tile([36, D], F16, tag="spat", bufs=3)
                cp4 = cpat[:].rearrange("j (k i) -> j k i", i=4)
                sp4 = spat[:].rearrange("j (k i) -> j k i", i=4)
                for i in range(4):
                    nc.vector.tensor_scalar(
                        out=cp4[:, :, i], in0=ones64[:],
                        scalar1=scal_bc[:, b * 8 + i:b * 8 + i + 1],
                        scalar2=None, op0=ALU.mult)
                    nc.gpsimd.tensor_scalar(
                        out=sp4[:, :, i], in0=ones64[:],
                        scalar1=scal_bc[:, b * 8 + 4 + i:b * 8 + 5 + i],
                        scalar2=None, op0=ALU.mult)
                # ---- rot = e*c + esw*s; fp8 hi + lo split ----
                t1 = bpool.tile([36, D], F16, tag="t1", bufs=3)
                nc.vector.tensor_mul(t1[:], e16[:], cpat[:])
                t2 = bpool.tile([36, D], F16, tag="t2", bufs=3)
                nc.gpsimd.tensor_mul(t2[:], esw[:], spat[:])
                rot = bpool.tile([36, D], F16, tag="rot", bufs=3)
                nc.vector.tensor_add(rot[:], t1[:], t2[:])
                lhsT = bpool.tile([36, 2, D], FP8, tag="lhsT", bufs=3)
                nc.scalar.activation(lhsT[:, 0, :], rot[:], AF.Copy)
                hi16 = bpool.tile([36, D], F16, tag="hi16", bufs=3)
                nc.scalar.activation(hi16[:], lhsT[:, 0, :], AF.Copy)
                nc.vector.tensor_tensor(out=lhsT[:, 1, :], in0=rot[:],
                                        in1=hi16[:], op=ALU.subtract)
                # ---- one-hot (fp8) ----
                oh = bpool.tile([36, N], FP8, tag="oh", bufs=3)
                nc.vector.tensor_scalar(out=oh[:],
                                        in0=flat_bc[:, b * N:(b + 1) * N],
                                        scalar1=iota_s[:], scalar2=None,
                                        op0=ALU.is_equal)
                oh_b = oh[:].unsqueeze(1).to_broadcast((36, 2, N))

                g = b % G
                if g == 0:
                    obuf = opool.tile([128, G * 2 * N], F16, tag="obuf", bufs=2)
                ob3 = obuf[:].rearrange("p (g h n) -> p g h n", g=G, h=2)
                # ---- transposed gather: 2 DoubleRow matmuls / batch ----
                for h in range(2):
                    ps = psum.tile([128, N], F32, tag="ps", bufs=6)
                    nc.tensor.matmul(ps[:], lhsT[:, :, h * 128:(h + 1) * 128],
                                     oh_b, start=True, stop=True, perf_mode=DR)
                    if h == 0:
                        nc.vector.tensor_copy(ob3[:, g, h, :], ps[:])
                    else:
                        nc.scalar.activation(ob3[:, g, h, :], ps[:], AF.Copy)
                if g == G - 1:
                    b0 = b - G + 1
                    nc.sync.dma_start(
                        out=out[:, b0 * 2 * N:(b0 + G) * 2 * N], in_=obuf[:])

    nc.compile()
    return nc


@functools.lru_cache(maxsize=1)
def _get_nc() -> bass.Bass:
    return build_bass()


def kernel_with_results(excitations, n_electrons, n_protons, emb_weight,
                        lookup_table, trace=False):
    exc = np.asarray(excitations)
    flat = (exc[..., 0] * 6 + exc[..., 1]).astype(np.int8).reshape(B, N)
    ne = np.asarray(n_electrons, dtype=np.float32)
    npr = np.asarray(n_protons, dtype=np.float32)
    emb = np.ascontiguousarray(np.asarray(emb_weight, dtype=np.float32))
    lut_f = np.ascontiguousarray(
        np.asarray(lookup_table).astype(np.float32).reshape(1, 36))

    in_maps = []
    for c in range(N_CORES):
        sl = slice(c * BL, (c + 1) * BL)
        in_maps.append({
            "flat": np.ascontiguousarray(flat[sl].reshape(1, BL * N)),
            "ne": np.ascontiguousarray(ne[sl].reshape(BL, 1)),
            "npr": np.ascontiguousarray(npr[sl].reshape(BL, 1)),
            "emb": emb,
            "lut": lut_f,
        })

    nc = _get_nc()
    res = run_bass_kernel_spmd(nc, in_maps, list(range(N_CORES)), trace=trace)
    shards = []
    for c in range(N_CORES):
        arr = np.asarray(res.results[c]["out"]).reshape(128, BL, 2, N)
        shards.append(arr.transpose(1, 3, 2, 0).reshape(BL, N, D))
    out_arr = np.concatenate(shards, axis=0).astype(np.float32)
    return np.ascontiguousarray(out_arr), res


def kernel(excitations, n_electrons, n_protons, emb_weight, lookup_table):
    out_arr, _ = kernel_with_results(excitations, n_electrons, n_protons,
                                     emb_weight, lookup_table)
    return out_arr
